# revision 29
# baseline (speedup 1.0000x reference)
"""CostVolume (9x9 correlation window + leaky_relu) for trn2, 8 NeuronCores.

Problem: x1, x2: [B=8, C=128, H=96, W=320] fp32
  out[b, 9*dy+dx, h, w] =
      leaky_relu(mean_c(x1[b,c,h,w] * x2pad[b,c,h+dy,w+dx]), 0.1)
with x2 zero-padded by 4 on both spatial axes.

Sharding: data-parallel over batch - one batch per core, SPMD over 8 cores.

Per-core pipeline (single TileContext, h-chunk pipelined):
  - inputs stream HBM->SBUF via gpsimd cast-DMA straight to bf16 (the cast is
    free in the SDMA datapath; fp32 never lands in SBUF).
  - channel contraction on TensorE as bf16 band matmuls: per (h, 32-px strip)
    one matmul with the 9 dy-window rows batched into a single 360-wide moving
    AP; 4 strips pack the 128x128 PE array via tile_position col groups into
    one PSUM bank [128, 360].
  - eviction on DVE: one scalar_tensor_tensor max(0.1*x, x) (= unscaled leaky)
    converting PSUM fp32 -> SBUF fp16, relayouting dy-minor so each pixel's
    window values are contiguous (t = 9*j + dy).
  - deskew (extract each pixel's 81 diagonal values from the band) via a DRAM
    fp16 scratch: band store is contiguous; the per-pixel diagonal is an
    affine gather on the DRAM side (stride 369 = 360+9), batched over 8 h rows.
  - PE transposes [px, 81] -> [81, px] (fp16 identity), ACT copy applies the
    1/128 mean scale and converts to fp32, one big store per 24-row chunk with
    the (dx,dy)->d=9*dy+dx permutation absorbed in the store AP.
"""

from contextlib import ExitStack

import numpy as np

import concourse.bass as bass
import concourse.mybir as mybir
import concourse.tile as tile
from concourse import masks
from concourse.ap import AP
from concourse.bass_utils import run_bass_kernel_spmd

F32 = mybir.dt.float32
F16 = mybir.dt.float16
BF16 = mybir.dt.bfloat16
C = 128
PAD = 4
D = 9
ND = D * D  # 81
WIN = 40  # 32 + 8: moving window per strip
BAND = D * WIN  # 360 band values per pixel in scratch

# The walrus build in this toolchain rejects instructions carrying more than
# one sync wait ("Too many sync wait commands"). Split: any instruction with
# >1 on_wait gets preceding same-engine NoOps carrying the extra waits
# (engine streams execute in order, so the gating is identical).
_MAX_WAITS = 1


def _split_sync_waits(nc):
    for bbname, bassbb in nc.bb_map.items():
        bb = bassbb.bb
        insts = list(bb.instructions)
        out = []
        changed = False
        for inst in insts:
            si = getattr(inst, "sync_info", None)
            waits = list(si.on_wait) if (si is not None and si.on_wait) else []
            if len(waits) > _MAX_WAITS:
                changed = True
                spill, keep = waits[:-_MAX_WAITS], waits[-_MAX_WAITS:]
                for k in range(0, len(spill), _MAX_WAITS):
                    chunk = spill[k : k + _MAX_WAITS]
                    out.append(
                        mybir.InstNoOp(
                            name=f"I-waitsplit-{nc.next_id()}",
                            engine=inst.engine,
                            ins=[],
                            outs=[],
                            sync_info=mybir.SyncInfo(on_wait=chunk, on_update=[]),
                        )
                    )
                si.on_wait = keep
            out.append(inst)
        if changed:
            try:
                bb.instructions[:] = out
            except TypeError:
                while len(bb.instructions):
                    bb.instructions.pop()
                for i in out:
                    bb.add_instruction(i)


def _th(t):
    return t.tensor if isinstance(t, AP) else t


def build_kernel(nc: bass.Bass, H=96, W=320, HB=12):
    assert H % HB == 0
    x1 = nc.declare_dram_parameter("x1", [C, H, W], F32, isOutput=False)
    x2 = nc.declare_dram_parameter("x2", [C, H, W], F32, isOutput=False)
    out = nc.declare_dram_parameter("out", [ND, H, W], F32, isOutput=True)

    # fp16 band scratch, w-major: addr = (w*H + h)*BAND + 9*j + dy
    scratch = nc.dram_tensor("scratch", [H * W * BAND], F16)
    sh = _th(scratch)
    oh = _th(out)

    W2 = W + 2 * PAD
    HF = H + 2 * PAD  # x2 resident rows (whole image + halo)
    wtiles = []
    wleft, w0 = W, 0
    while wleft > 0:
        mw = min(128, wleft)
        wtiles.append((w0, mw))
        w0 += mw
        wleft -= mw
    nblocks = H // HB

    with tile.TileContext(nc) as tc, ExitStack() as ctx:
        const_pool = ctx.enter_context(tc.tile_pool(name="const", bufs=1))
        x2_pool = ctx.enter_context(tc.tile_pool(name="x2p", bufs=1))
        x1_pool = ctx.enter_context(tc.tile_pool(name="x1p", bufs=3))
        stg_pool = ctx.enter_context(tc.tile_pool(name="stg", bufs=4))
        rl_pool = ctx.enter_context(tc.tile_pool(name="rl", bufs=2))
        g_pool = ctx.enter_context(tc.tile_pool(name="g", bufs=2))
        o_pool = ctx.enter_context(tc.tile_pool(name="o", bufs=2))
        ps_pool = ctx.enter_context(tc.tile_pool(name="ps", bufs=3, space="PSUM"))
        tp_pool = ctx.enter_context(tc.tile_pool(name="tp", bufs=2, space="PSUM"))

        ident = const_pool.tile([128, 128], F16)
        masks.make_identity(nc, ident[:])
        idh = _th(ident)

        # x2 resident for the whole image, zero-padded halo
        x2c = x2_pool.tile([C, HF * W2], BF16, tag="x2c", name="x2c")
        x2h = _th(x2c)
        nc.vector.memset(AP(x2h, 0, [[HF * W2, C], [W2, HF], [1, PAD]]), 0.0)
        nc.vector.memset(AP(x2h, PAD + W, [[HF * W2, C], [W2, HF], [1, PAD]]), 0.0)
        nc.vector.memset(AP(x2h, PAD, [[HF * W2, C], [W2, PAD], [1, W]]), 0.0)
        nc.vector.memset(
            AP(x2h, (H + PAD) * W2 + PAD, [[HF * W2, C], [W2, PAD], [1, W]]), 0.0
        )

        def load_x2(a, b2):  # x2 rows [a, b2) -> tile rows [a+PAD, b2+PAD)
            nc.gpsimd.dma_start(
                out=AP(
                    x2h, (a + PAD) * W2 + PAD, [[HF * W2, C], [W2, b2 - a], [1, W]]
                ),
                in_=AP(_th(x2), a * W, [[H * W, C], [1, (b2 - a) * W]]),
            )

        def load_x1(b):  # x1 rows for block b
            x1b = x1_pool.tile([C, HB * W], BF16, tag="x1b", name="x1b")
            nc.gpsimd.dma_start(
                out=x1b[:],
                in_=AP(_th(x1), b * HB * W, [[H * W, C], [1, HB * W]]),
            )
            return _th(x1b)

        # x2 load pieces: block b newly needs rows [HB*b+4, HB*b+16) (b>=1)
        load_x2(0, HB + PAD)
        x1hs = {0: load_x1(0)}

        pending = None

        def flush_units(blk, hh_list):
            b, gbufs, h0b, osh = blk
            for hh in hh_list:
                tp = tp_pool.tile([128, W], F16, tag="tp")
                tph = _th(tp)
                for wt, (w0, mw) in enumerate(wtiles):
                    gh = _th(gbufs[wt])
                    nc.tensor.transpose(
                        AP(tph, w0, [[W, ND], [1, mw]]),
                        AP(gh, hh * ND, [[HB * ND, mw], [1, ND]]),
                        AP(idh, 0, [[128, mw], [1, mw]]),
                    )
                nc.vector.tensor_copy(
                    out=AP(osh, hh * W, [[HB * W, ND], [1, W]]),
                    in_=AP(tph, 0, [[W, ND], [1, W]]),
                )

        def flush_store(blk):
            b, gbufs, h0b, osh = blk
            # per-block store; (dx,dy) -> d=9*dy+dx via AP dims
            nc.sync.dma_start(
                out=AP(
                    oh, h0b * W, [[H * W, D], [D * H * W, D], [1, HB * W]]
                ),
                in_=AP(osh, 0, [[HB * W, ND], [1, HB * W]]),
            )

        for b in range(nblocks):
            h0b = b * HB
            x1h = x1hs.pop(b)
            # per-(block, wtile) fp16 band staging: [pixel, hh*360 + 9j+dy]
            stgs = [
                stg_pool.tile([128, HB * BAND], F16, tag="stg", name=f"stg{wt}")
                for wt in range(len(wtiles))
            ]
            for hp in range(HB // 2):  # h pairs share a 2-bank psum tile
                if pending is not None:
                    flush_units(pending, [2 * hp, 2 * hp + 1])
                for wt, (w0, mw) in enumerate(wtiles):
                    nstrip = mw // 32
                    ps = ps_pool.tile([128, 1024], F32, tag="band")
                    ph = _th(ps)
                    for sub in range(2):
                        hh = 2 * hp + sub
                        r = h0b + hh  # x2 tile row base = r (halo built in)
                        for s in range(nstrip):
                            wl = w0 + 32 * s
                            nc.tensor.matmul(
                                AP(
                                    ph,
                                    32 * s * 1024 + 512 * sub,
                                    [[1024, 32], [1, D], [D, WIN]],
                                ),
                                AP(x1h, hh * W + wl, [[HB * W, C], [1, 32]]),
                                AP(
                                    x2h,
                                    r * W2 + wl,
                                    [[HF * W2, C], [W2, D], [1, WIN]],
                                ),
                                start=True,
                                stop=True,
                                tile_position=(0, 32 * s),
                            )
                    # paired leaky eviction for both h rows, contiguous:
                    # stg[p, (2hp+sub)*360+t] = leaky_0.1(ps[p, 512*sub+t]/C)
                    sth = _th(stgs[wt])
                    if wt == 1 and hp % 2 == 1:
                        relu_t = rl_pool.tile(
                            [128, 2 * BAND], F16, tag="relu_t", name="relu_t"
                        )
                        rth = _th(relu_t)
                        nc.vector.tensor_scalar(
                            out=AP(rth, 0, [[2 * BAND, mw], [BAND, 2], [1, BAND]]),
                            in0=AP(ph, 0, [[1024, mw], [512, 2], [1, BAND]]),
                            scalar1=0.9 / C,
                            scalar2=0.0,
                            op0=mybir.AluOpType.mult,
                            op1=mybir.AluOpType.max,
                        )
                        nc.vector.scalar_tensor_tensor(
                            out=AP(
                                sth,
                                2 * hp * BAND,
                                [[HB * BAND, mw], [BAND, 2], [1, BAND]],
                            ),
                            in0=AP(ph, 0, [[1024, mw], [512, 2], [1, BAND]]),
                            scalar=0.1 / C,
                            in1=AP(rth, 0, [[2 * BAND, mw], [BAND, 2], [1, BAND]]),
                            op0=mybir.AluOpType.mult,
                            op1=mybir.AluOpType.add,
                        )
                    else:
                        nc.scalar.activation(
                            out=AP(
                                sth,
                                2 * hp * BAND,
                                [[HB * BAND, mw], [BAND, 2], [1, BAND]],
                            ),
                            in_=AP(ph, 0, [[1024, mw], [512, 2], [1, BAND]]),
                            func=mybir.ActivationFunctionType.Prelu,
                            scale=1.0 / C,
                            alpha=0.1,
                        )
            # band stores: strips 1.. only (strip 0 deskews on-chip)
            for wt, (w0, mw) in enumerate(wtiles):
                sth = _th(stgs[wt])
                nc.sync.dma_start(
                    out=AP(
                        sh,
                        ((w0 + 32) * H + h0b) * BAND,
                        [[H * BAND, mw - 32], [1, HB * BAND]],
                    ),
                    in_=AP(
                        sth,
                        32 * HB * BAND,
                        [[HB * BAND, mw - 32], [1, HB * BAND]],
                    ),
                )
            # deskew: strip 0 SBUF->SBUF diagonal; strips 1.. DRAM gather
            gbufs = []
            for wt, (w0, mw) in enumerate(wtiles):
                gbuf = g_pool.tile([128, HB * ND], F16, tag=f"gb{wt}")
                gh = _th(gbuf)
                nc.scalar.dma_start(
                    out=AP(gh, 0, [[HB * ND, 32], [ND, HB], [1, ND]]),
                    in_=AP(
                        _th(stgs[wt]),
                        0,
                        [[HB * BAND + D, 32], [BAND, HB], [1, ND]],
                    ),
                )
                for s in range(1, mw // 32):
                    wl = w0 + 32 * s
                    nc.sync.dma_start(
                        out=AP(
                            gh,
                            (32 * s) * (HB * ND),
                            [[HB * ND, 32], [ND, HB], [1, ND]],
                        ),
                        in_=AP(
                            sh,
                            (wl * H + h0b) * BAND,
                            [[H * BAND + D, 32], [BAND, HB], [1, ND]],
                        ),
                    )
                gbufs.append(gbuf)
            # prefetch next block's inputs
            if b + 1 < nblocks:
                load_x2(HB * (b + 1) + PAD, min(HB * (b + 2) + PAD, H))
                x1hs[b + 1] = load_x1(b + 1)
            if pending is not None:
                flush_store(pending)
            ostg = o_pool.tile([128, HB * W], F32, tag="ostg", name="ostg")
            pending = (b, gbufs, h0b, _th(ostg))

        flush_units(pending, list(range(HB)))
        flush_store(pending)

    return nc


_COMPILED = {}


def _build():
    key = "cv"
    if key not in _COMPILED:
        nc = bass.Bass()
        build_kernel(nc)
        _split_sync_waits(nc)
        _COMPILED[key] = nc
    return _COMPILED[key]


def kernel(**inputs) -> np.ndarray:
    x1 = np.asarray(inputs["x1"], dtype=np.float32)
    x2 = np.asarray(inputs["x2"], dtype=np.float32)
    B = x1.shape[0]
    nc = _build()
    core_ids = list(range(8))
    in_maps = [
        {"x1": np.ascontiguousarray(x1[b]), "x2": np.ascontiguousarray(x2[b])}
        for b in range(B)
    ]
    res = run_bass_kernel_spmd(nc, in_maps, core_ids)
    return np.stack([np.asarray(res.results[b]["out"]) for b in range(B)], axis=0)
